# revision 4
# baseline (speedup 1.0000x reference)
"""AttentionPooling (segment softmax + weighted segment sum) on 8 trn2 cores.

Strategy: shard whole segments across cores (sorted batch -> contiguous node
ranges), pad each core's slice to a common node count, run one SPMD Bass/Tile
program.  HBM traffic is minimized by uploading x twice in bf16 from the host:
node-major ``x`` [nmax, 264] (ch 0-255 = features, ch 256 = 1.0 for the
denominator, 257-263 pad) feeding the weighted-sum matmul as the moving
operand, and channel-major ``xt`` [2, 128, nmax] feeding the MLP matmul.
No on-chip cast, transpose, or DRAM bounce.  Per 2048-node chunk: two HWDGE
loads (SP + ACT rings), PE matmuls for the MLP score (hidden-partitioned),
exp on ACT, onehot(segment)*e built on DVE, then per 128-node tile a
stationary-swap pair: score column (h-tile x W2) and weighted-sum
(we-tile x [x|1]) accumulating [64 segs, 257] in PSUM.  Softmax
max-subtraction is skipped: |s| <= ||W2||_1 + |b2| ~ 28, exp stays in fp32
range.  bf16 numerics match the previous on-device-cast version.
"""

from contextlib import ExitStack

import numpy as np
import ml_dtypes

import concourse.bass as bass
import concourse.bacc as bacc
import concourse.tile as tile
from concourse import mybir
from concourse.bass_utils import run_bass_kernel_spmd

N_CORES = 8
NUM_GRAPHS = 512
SEGS_PER_CORE = NUM_GRAPHS // N_CORES  # 64
D = 256          # in channels
DA = 264         # in channels + ones col + pad (16B-aligned rows)
H = 128          # hidden
P = 128          # partitions
TILE_N = 128     # nodes per weight tile
CHUNK_T = 16     # tiles per chunk
CHUNK_N = TILE_N * CHUNK_T  # 2048 nodes per chunk

_BF16 = mybir.dt.bfloat16
_F32 = mybir.dt.float32
_I32 = mybir.dt.int32


def _build_program(n_chunks: int, b2_val: float):
    nc = bacc.Bacc()
    nmax = n_chunks * CHUNK_N
    nt = nmax // TILE_N

    x_d = nc.declare_dram_parameter("x", [nmax, DA], _BF16, isOutput=False)
    xt_d = nc.declare_dram_parameter("xt", [2, P, nmax], _BF16, isOutput=False)
    bt_d = nc.declare_dram_parameter("batch_t", [P, nt + SEGS_PER_CORE], _I32, isOutput=False)
    w1_d = nc.declare_dram_parameter("w1", [D, H], _BF16, isOutput=False)
    w2_d = nc.declare_dram_parameter("w2", [H, 1], _BF16, isOutput=False)
    b1_d = nc.declare_dram_parameter("b1", [H, 1], _F32, isOutput=False)
    out_d = nc.declare_dram_parameter("out_g", [SEGS_PER_CORE, D], _F32, isOutput=True)

    # chunked views: chunk c -> (p=node%128, t=tile-in-chunk, ch)
    x_ap = x_d[:].rearrange("(c t p) ch -> c p t ch", p=P, t=CHUNK_T)
    xt_ap = xt_d[:].rearrange("h p (c n) -> c p h n", n=CHUNK_N)

    with tile.TileContext(nc) as tc, ExitStack() as ctx:
        const_pool = ctx.enter_context(tc.tile_pool(name="consts", bufs=1))
        x_pool = ctx.enter_context(tc.tile_pool(name="x", bufs=3))
        xt_pool = ctx.enter_context(tc.tile_pool(name="xt", bufs=3))
        h_pool = ctx.enter_context(tc.tile_pool(name="h", bufs=2))
        we_pool = ctx.enter_context(tc.tile_pool(name="we", bufs=3))
        ecol_pool = ctx.enter_context(tc.tile_pool(name="ecol", bufs=2))
        fin_pool = ctx.enter_context(tc.tile_pool(name="fin", bufs=1))
        psum_h = ctx.enter_context(
            tc.tile_pool(name="psum_h", bufs=2, space=bass.MemorySpace.PSUM))
        psum_s = ctx.enter_context(
            tc.tile_pool(name="psum_s", bufs=2, space=bass.MemorySpace.PSUM))
        psum_acc = ctx.enter_context(
            tc.tile_pool(name="psum_acc", bufs=1, space=bass.MemorySpace.PSUM))

        # ---- constants / weights ----
        w1_sb = const_pool.tile([P, 2, H], _BF16, tag="w1")   # [:, 0, :]=ch 0-127
        nc.sync.dma_start(w1_sb[:, 0, :], w1_d[0:128, :])
        nc.sync.dma_start(w1_sb[:, 1, :], w1_d[128:256, :])
        w2_sb = const_pool.tile([P, 1], _BF16, tag="w2")
        nc.sync.dma_start(w2_sb[:], w2_d[:])
        b1_sb = const_pool.tile([P, 1], _F32, tag="b1")
        nc.sync.dma_start(b1_sb[:], b1_d[:])
        bt_sb = const_pool.tile([P, nt + SEGS_PER_CORE], _I32, tag="bt")
        nc.sync.dma_start(bt_sb[:], bt_d[:])
        iota_sb = bt_sb[:, nt:nt + SEGS_PER_CORE]

        acc_ps = psum_acc.tile([SEGS_PER_CORE, DA], _F32, tag="acc")

        saved = {}

        def emit_score(c):
            x_sb = x_pool.tile([P, CHUNK_T, DA], _BF16, tag="x")
            nc.sync.dma_start(x_sb[:], x_ap[c])
            xt_sb = xt_pool.tile([P, 2, CHUNK_N], _BF16, tag="xt")
            nc.scalar.dma_start(xt_sb[:], xt_ap[c])

            # h = tanh(x @ W1 + b1), hidden-partitioned, bf16
            h_bf = h_pool.tile([P, CHUNK_N], _BF16, tag="h")
            for s in range(CHUNK_N // 512):
                ph = psum_h.tile([P, 512], _F32, tag="ph")
                sl = slice(s * 512, (s + 1) * 512)
                nc.tensor.matmul(ph[:], w1_sb[:, 0, :], xt_sb[:, 0, sl],
                                 start=True, stop=False)
                nc.tensor.matmul(ph[:], w1_sb[:, 1, :], xt_sb[:, 1, sl],
                                 start=False, stop=True)
                nc.scalar.activation(h_bf[:, sl], ph[:],
                                     mybir.ActivationFunctionType.Tanh,
                                     bias=b1_sb[:])
            saved[c] = (x_sb, h_bf)

        def emit_scores_and_wsum(c):
            # score columns for chunk c interleaved with weighted-sum matmuls
            # for chunk c-1: the 257-col wsum matmuls hide the score
            # LDWEIGHTS-bound pairs.
            x_sb, h_bf = saved.pop(c)
            prev = saved.pop(("w", c - 1), None)
            ps_s = psum_s.tile([P, CHUNK_T], _F32, tag="ps_s")
            first_wsum = c == 1  # prev chunk 0 opens the accumulation group
            for t in range(CHUNK_T):
                nc.tensor.matmul(ps_s[:, t:t + 1],
                                 h_bf[:, t * TILE_N:(t + 1) * TILE_N],
                                 w2_sb, start=True, stop=True)
                if prev is not None:
                    we_p, x_p = prev
                    nc.tensor.matmul(acc_ps[:], we_p[:, t, :],
                                     x_p[:, t, 0:DA],
                                     start=(first_wsum and t == 0),
                                     stop=False,
                                     skip_group_check=True)

            # e = exp(s + b2)  (node-partitioned, fp32)
            e_col = ecol_pool.tile([P, CHUNK_T], _F32, tag="ecol")
            nc.scalar.activation(e_col[:], ps_s[:],
                                 mybir.ActivationFunctionType.Exp,
                                 bias=float(b2_val))

            # we[p, t, g] = (batch_t == g) * e   (bf16)
            cmp = we_pool.tile([P, CHUNK_T, SEGS_PER_CORE], _BF16, tag="cmp")
            bt_c = bt_sb[:, c * CHUNK_T:(c + 1) * CHUNK_T]
            nc.vector.tensor_tensor(
                cmp[:],
                bt_c.unsqueeze(2).broadcast_to([P, CHUNK_T, SEGS_PER_CORE]),
                iota_sb.unsqueeze(1).broadcast_to([P, CHUNK_T, SEGS_PER_CORE]),
                mybir.AluOpType.is_equal)
            we = we_pool.tile([P, CHUNK_T, SEGS_PER_CORE], _BF16, tag="we")
            nc.vector.tensor_tensor(
                we[:], cmp[:],
                e_col[:].unsqueeze(2).broadcast_to([P, CHUNK_T, SEGS_PER_CORE]),
                mybir.AluOpType.mult)
            saved[("w", c)] = (we, x_sb)

        def emit_final_wsum(c):
            we_p, x_p = saved.pop(("w", c))
            for t in range(CHUNK_T):
                nc.tensor.matmul(acc_ps[:], we_p[:, t, :], x_p[:, t, 0:DA],
                                 start=(n_chunks == 1 and t == 0),
                                 stop=(t == CHUNK_T - 1),
                                 skip_group_check=True)

        emit_score(0)
        for c in range(n_chunks):
            if c + 1 < n_chunks:
                emit_score(c + 1)
            emit_scores_and_wsum(c)
        emit_final_wsum(n_chunks - 1)

        # ---- epilogue: out = acc[:, 0:256] / acc[:, 256] ----
        den_sb = fin_pool.tile([SEGS_PER_CORE, 1], _F32, tag="den_sb")
        nc.vector.tensor_scalar_add(den_sb[:], acc_ps[:, D:D + 1], 1e-30)
        rec_sb = fin_pool.tile([SEGS_PER_CORE, 1], _F32, tag="rec_sb")
        nc.vector.reciprocal(rec_sb[:], den_sb[:])
        out_sb = fin_pool.tile([SEGS_PER_CORE, D], _F32, tag="out_sb")
        nc.vector.tensor_scalar_mul(out_sb[:], acc_ps[:, 0:D], rec_sb[:])
        nc.sync.dma_start(out_d[:], out_sb[:])

    return nc


def _prepare_inputs(x, W1, b1, W2, b2, batch):
    batch = np.asarray(batch).astype(np.int64)
    # core k owns segments [64k, 64(k+1)); sorted batch -> contiguous ranges
    bounds = np.searchsorted(batch, np.arange(0, NUM_GRAPHS + 1, SEGS_PER_CORE))
    counts = np.diff(bounds)
    nmax = int(np.max(counts))
    n_chunks = max(1, (nmax + CHUNK_N - 1) // CHUNK_N)
    nmax_pad = n_chunks * CHUNK_N

    x_bf = np.asarray(x, np.float32).astype(ml_dtypes.bfloat16)
    w1_bf = np.asarray(W1, np.float32).astype(ml_dtypes.bfloat16)
    w2_bf = np.asarray(W2, np.float32).reshape(H, 1).astype(ml_dtypes.bfloat16)
    b1_col = np.asarray(b1, np.float32).reshape(H, 1)

    in_maps = []
    for k in range(N_CORES):
        lo, hi = int(bounds[k]), int(bounds[k + 1])
        cnt = hi - lo
        x_pad = np.zeros((nmax_pad, DA), ml_dtypes.bfloat16)
        x_pad[:cnt, 0:D] = x_bf[lo:hi]
        x_pad[:, D] = ml_dtypes.bfloat16(1.0)
        xt_pad = np.zeros((2, P, nmax_pad), ml_dtypes.bfloat16)
        xt_pad[:, :, :cnt] = x_bf[lo:hi].T.reshape(2, P, cnt)
        bt = np.full((nmax_pad,), -1, np.int32)
        bt[:cnt] = batch[lo:hi] - k * SEGS_PER_CORE
        bt_t = bt.reshape(nmax_pad // P, P).T  # (128, nt)
        iota_cols = np.tile(np.arange(SEGS_PER_CORE, dtype=np.int32), (P, 1))
        bt_t = np.concatenate([bt_t, iota_cols], axis=1).copy()
        in_maps.append({
            "x": x_pad,
            "xt": xt_pad,
            "batch_t": bt_t,
            "w1": w1_bf,
            "w2": w2_bf,
            "b1": b1_col,
        })
    return in_maps, n_chunks


def run(x, W1, b1, W2, b2, batch, trace=False, trace_kwargs=None):
    in_maps, n_chunks = _prepare_inputs(x, W1, b1, W2, b2, batch)
    nc = _build_program(n_chunks, float(np.asarray(b2).reshape(-1)[0]))
    nc.finalize()
    res = run_bass_kernel_spmd(nc, in_maps, list(range(N_CORES)),
                               trace=trace, **(trace_kwargs or {}))
    out = np.concatenate([np.asarray(res.results[k]["out_g"], np.float32)
                          for k in range(N_CORES)], axis=0)
    return out, res


def kernel(x, W1, b1, W2, b2, batch):
    out, _ = run(x, W1, b1, W2, b2, batch)
    return out
